# revision 14
# baseline (speedup 1.0000x reference)
"""MLA (multi-head latent attention) prefill kernel for 8 Trainium2 NeuronCores.

Tensor-parallel across heads: each of the 8 cores owns NH/8 = 2 heads.
wq / wkv_b output dims and the wo input dim are sharded by head; the post-wo
partial sums are reduced on the host (the unshard step of a RowParallelLinear).

The shared latent-KV projection (wkv_a) + rms-norm is NOT replicated: each
core computes it for a 256-token slice of the sequence (both batches fused
into one 512-column pass) and the normalized latent (+ rope'd k_pe) is
exchanged with a single on-device AllGather, cutting the dominant phase-A
matmul volume 8x versus the replicated form.

Everything on-device runs in a transposed [feature, seq] layout so that
attention scores come out as S^T[sk, sq]; the softmax reductions over the
key axis (= partitions) are done with ones-vector matmuls (denominator) and
a gpsimd partition_all_reduce (rms-norm sum of squares), so the kernel needs
no on-chip transposes. Max-subtraction is skipped (logits are O(10), exp is
safe in fp32). All matmul operands are bf16 (full PE rate, half the
LDWEIGHTS + SBUF + DMA cost of fp32r); PSUM accumulation stays fp32.
Reciprocals are taken after broadcasting to 128 partitions (a [1,N]
single-lane reciprocal is ~6x slower than a [128,N] one).

Scheduling notes: phase-Q input DMA is prefetched before the collective
competes for DMA bandwidth; softmax den/out matmuls run one kt behind the
exp that feeds them; each chunk's wo projection is deferred into the next
chunk's score matmuls so its normalize chain never stalls the PE.

Host-side prep: weights are pre-permuted so rope pairs are de-interleaved
([even | odd] blocks) and head blocks land on clean 128-partition tiles;
the 1/sqrt(d_qk) scale and kv_norm weight are folded into wq / wkv_b.
"""

import sys

sys.path.insert(0, "/opt/trn_rl_repo")

from contextlib import ExitStack

import numpy as np
import ml_dtypes

import concourse.tile as tile
from concourse import bacc, mybir
from concourse import bass_utils
from concourse import bass_isa

B, S, DIM = 2, 2048, 2048
NH = 16
D_NOPE, D_ROPE, D_V = 128, 64, 128
D_QK = D_NOPE + D_ROPE  # 192
KV_RANK = 512
RMS_EPS = 1e-6
N_CORES = 8
HPC = NH // N_CORES  # heads per core = 2

F32 = mybir.dt.float32
F32R = mybir.dt.float32r
BF16 = mybir.dt.bfloat16
F16 = mybir.dt.float16
EXP = mybir.ActivationFunctionType.Exp
SQRT = mybir.ActivationFunctionType.Sqrt

NPBF16 = ml_dtypes.bfloat16

CH = 512            # wq / kn-v seq chunk (moving N of projection matmuls)
SLC = S // N_CORES  # 256: per-core wkv_a token slice
SL2 = B * SLC       # 512: both batches' slices fused into one pass
SQC = 512           # phase-B query chunk
N_DT = DIM // 128   # 16 k-tiles over model dim
N_RT = KV_RANK // 128  # 4 k-tiles over kv rank
N_KT = S // 128     # 16 key tiles

# stream_shuffle permutes WITHIN each 32-partition quadrant (same mask per
# quadrant), so rope pairs are packed [even(16) | odd(16)] per quadrant and the
# shuffle swaps the 16-row halves.
SHUF_Q = list(range(16, 32)) + list(range(16))
SHUF_K = SHUF_Q

# row permutation packing a 64-row interleaved rope block into that layout:
# pair i -> even at 32*(i//16) + i%16, odd at 32*(i//16) + 16 + i%16
_IDX64 = [0] * 64
for _i in range(32):
    _IDX64[32 * (_i // 16) + (_i % 16)] = 2 * _i
    _IDX64[32 * (_i // 16) + 16 + (_i % 16)] = 2 * _i + 1

_cache = {}
last_results = None


def _build(mask_mode):
    nc = bacc.Bacc("TRN2", target_bir_lowering=False, debug=False, num_devices=N_CORES)

    xT = nc.dram_tensor("xT", [B, N_DT, 128, S], BF16, kind="ExternalInput").ap()
    xsT = nc.dram_tensor("xsT", [B, N_DT, 128, SLC], BF16, kind="ExternalInput").ap()
    wqT = nc.dram_tensor("wqT", [128, N_DT, 384], BF16, kind="ExternalInput").ap()
    wkaT = nc.dram_tensor("wkaT", [128, N_DT, 576], BF16, kind="ExternalInput").ap()
    wkbT = nc.dram_tensor("wkbT", [128, N_RT, 512], BF16, kind="ExternalInput").ap()
    woT = nc.dram_tensor("woT", [128, HPC, DIM], BF16, kind="ExternalInput").ap()
    ropeA = nc.dram_tensor("ropeA", [128, S], F32R, kind="ExternalInput").ap()
    ropeB = nc.dram_tensor("ropeB", [128, S], F32R, kind="ExternalInput").ap()
    ropeSA = nc.dram_tensor("ropeSA", [64, SL2], F32R, kind="ExternalInput").ap()
    ropeSB = nc.dram_tensor("ropeSB", [64, SL2], F32R, kind="ExternalInput").ap()
    tri16 = nc.dram_tensor("tri16", [128, 128], BF16, kind="ExternalInput").ap()
    emaskT = None
    if mask_mode == "general":
        emaskT = nc.dram_tensor("emaskT", [N_KT, 128, S], BF16, kind="ExternalInput").ap()
    o = nc.dram_tensor("o", [B, DIM, S], F16, kind="ExternalOutput").ap()

    with tile.TileContext(nc) as tc:
        with ExitStack() as ctx, \
                nc.allow_low_precision(reason="bf16 matmul pipeline"):
            _body(ctx, tc, mask_mode, xT, xsT, wqT, wkaT, wkbT, woT,
                  ropeA, ropeB, ropeSA, ropeSB, tri16, emaskT, o)
    nc.compile()
    return nc


def _body(ctx, tc, mask_mode, xT, xsT, wqT, wkaT, wkbT, woT,
          ropeA, ropeB, ropeSA, ropeSB, tri16, emaskT, o):
    nc = tc.nc

    singles = ctx.enter_context(tc.tile_pool(name="singles", bufs=1))
    wq_s = singles.tile([128, N_DT, 384], BF16)
    nc.sync.dma_start(out=wq_s, in_=wqT)
    wka_s = singles.tile([128, N_DT, 576], BF16)
    nc.sync.dma_start(out=wka_s, in_=wkaT)
    wkb_s = singles.tile([128, N_RT, 512], BF16)
    nc.sync.dma_start(out=wkb_s, in_=wkbT)
    wo_s = singles.tile([128, HPC, DIM], BF16)
    nc.sync.dma_start(out=wo_s, in_=woT)
    cst = singles.tile([128, 128], BF16)
    nc.sync.dma_start(out=cst, in_=tri16)
    ones_col = cst[:, 127:128]   # [128,1] lhsT -> partition sum
    ones_row = cst[0:1, 0:128]   # [1,128] lhsT -> partition broadcast
    tri = cst[:, 0:128]          # keep-mask: 1 where free >= part
    rsa_s = singles.tile([64, SL2], F32R)
    nc.sync.dma_start(out=rsa_s, in_=ropeSA)
    rsb_s = singles.tile([64, SL2], F32R)
    nc.sync.dma_start(out=rsb_s, in_=ropeSB)
    epsb = singles.tile([128, 1], F32)
    nc.vector.memset(epsb, RMS_EPS)

    batchp = ctx.enter_context(tc.tile_pool(name="batchp", bufs=2))
    dramp = ctx.enter_context(tc.tile_pool(name="dramp", bufs=1, space="DRAM"))
    # xc pool opened before phase S so phase-Q input DMA is prefetched before
    # the collective starts competing for DMA bandwidth; closed after phase Q
    pq_stack = ExitStack()
    pq = pq_stack.enter_context(tc.tile_pool(name="pq", bufs=3))
    pq1 = pq_stack.enter_context(tc.tile_pool(name="pq1", bufs=2))

    all_chunks = [(b, ci) for b in range(B) for ci in range(S // CH)]

    def xc_dma(b, ci):
        t = pq.tile([128, N_DT, CH], BF16, tag="xc", bufs=3, name=f"xc{b}{ci}")
        nc.sync.dma_start(
            out=t, in_=xT[b, :, :, ci * CH:(ci + 1) * CH].rearrange("t p s -> p t s"))
        return t

    xc_pref = [xc_dma(*all_chunks[0]), xc_dma(*all_chunks[1])]

    # ========== Phase S: latent-kv slice (both batches fused) + gather =====
    lat_in = dramp.tile([B, 576, SLC], BF16, tag="lat_in")
    lat_g = dramp.tile([N_CORES, B, 576, SLC], BF16, tag="lat_g",
                       addr_space="Shared")

    with tc.tile_pool(name="psl", bufs=1) as psl, \
         tc.tile_pool(name="pslps", bufs=1, space="PSUM") as pslps:
        xs = psl.tile([128, N_DT, SL2], BF16)
        for b in range(B):
            nc.sync.dma_start(
                out=xs[:, :, b * SLC:(b + 1) * SLC],
                in_=xsT[b].rearrange("t p s -> p t s"))
        accs = [pslps.tile([128, SL2], F32, tag=f"acc{i}", bufs=1, name=f"acc{i}")
                for i in range(N_RT)]
        acc_pe = pslps.tile([64, SL2], F32, tag="accpe", bufs=1)
        for dt in range(N_DT):
            st, sp = dt == 0, dt == N_DT - 1
            for m in range(N_RT):
                nc.tensor.matmul(accs[m], wka_s[:, dt, m * 128:(m + 1) * 128],
                                 xs[:, dt, :], start=st, stop=sp)
            nc.tensor.matmul(acc_pe, wka_s[:, dt, 512:576],
                             xs[:, dt, :], start=st, stop=sp)

        # move latent out of PSUM immediately (scalar) so the acc banks free
        # up for phase Q's accumulators without waiting on the norm chain
        kvl = psl.tile([128, N_RT, SL2], BF16)
        for m in range(N_RT):
            nc.scalar.copy(kvl[:, m, :], accs[m])
        # k_pe rope (64 rows) — independent of the rms norm
        ktmp = psl.tile([64, SL2], F32)
        kpe16 = psl.tile([64, SL2], BF16)
        nc.vector.stream_shuffle(ktmp, acc_pe, SHUF_K)
        nc.vector.tensor_mul(ktmp, ktmp, rsb_s)
        nc.vector.tensor_mul(kpe16, acc_pe, rsa_s)
        nc.vector.tensor_add(kpe16, kpe16, ktmp)

        # rms-norm: sum of squares across the 512 latent rows via a
        # ones-matmul partition reduce, then a broadcast so the reciprocal
        # runs on all 128 lanes (ss and bcd share one PSUM bank via WAR)
        sq = psl.tile([128, N_RT, SL2], BF16)
        for m in range(N_RT):
            nc.scalar.square(sq[:, m, :], kvl[:, m, :])
        ss = pslps.tile([1, SL2], F32, tag="red", bufs=1, name="ss")
        for m in range(N_RT):
            nc.tensor.matmul(ss, ones_col, sq[:, m, :],
                             start=(m == 0), stop=(m == N_RT - 1))
        mrow = psl.tile([1, SL2], BF16)
        nc.scalar.activation(mrow, ss, SQRT, bias=epsb[0:1, :],
                             scale=1.0 / KV_RANK)
        bcd = pslps.tile([128, SL2], F32, tag="red", bufs=1, name="bcd")
        nc.tensor.matmul(bcd, ones_row, mrow, start=True, stop=True)
        rsb = psl.tile([128, SL2], F32R)
        nc.vector.reciprocal(rsb, bcd)
        latn = psl.tile([128, N_RT, SL2], BF16)
        for m in range(N_RT):
            nc.vector.tensor_mul(latn[:, m, :], kvl[:, m, :], rsb)

        for b in range(B):
            nc.sync.dma_start(
                out=lat_in[b, 0:512, :].rearrange("(r p) s -> p r s", p=128),
                in_=latn[:, :, b * SLC:(b + 1) * SLC])
            nc.sync.dma_start(out=lat_in[b, 512:576, :],
                              in_=kpe16[:, b * SLC:(b + 1) * SLC])
    nc.gpsimd.collective_compute(
        "AllGather",
        mybir.AluOpType.bypass,
        replica_groups=[list(range(N_CORES))],
        ins=[lat_in.opt()],
        outs=[lat_g.opt()],
    )

    # ================= Phase Q: query projection =================
    qTs, qpe1s = [], []
    with tc.tile_pool(name="pqps", bufs=6, space="PSUM") as pqps:
        qT = None
        for idx, (b, ci) in enumerate(all_chunks):
            if ci == 0:
                qT = batchp.tile([128, 3, S], BF16, tag="qT", name=f"qT{b}")
                qTs.append(qT)
            c0 = ci * CH
            xc = xc_pref[idx]
            if idx + 2 < len(all_chunks):
                xc_pref.append(xc_dma(*all_chunks[idx + 2]))
            ra = pq1.tile([128, CH], F32R, tag="ra", name=f"ra{b}{ci}")
            nc.sync.dma_start(out=ra, in_=ropeA[:, c0:c0 + CH])
            rb = pq1.tile([128, CH], F32R, tag="rb", name=f"rb{b}{ci}")
            nc.sync.dma_start(out=rb, in_=ropeB[:, c0:c0 + CH])

            accs = [pqps.tile([128, CH], F32, tag="wq", name=f"wq{i}")
                    for i in range(3)]
            for dt in range(N_DT):
                st, sp = dt == 0, dt == N_DT - 1
                for m in range(3):
                    nc.tensor.matmul(accs[m], wq_s[:, dt, m * 128:(m + 1) * 128],
                                     xc[:, dt, :], start=st, stop=sp)
            for m in range(2):
                nc.vector.tensor_copy(qT[:, m, c0:c0 + CH], accs[m])
            qpe = qT[:, 2, c0:c0 + CH]
            qtmp = pq1.tile([128, CH], F32, tag="qtmp", name=f"qtmp{b}{ci}")
            nc.vector.stream_shuffle(qtmp, accs[2], SHUF_Q)  # [o|e] swapped
            nc.vector.tensor_mul(qtmp, qtmp, rb)             # +/- sin terms
            nc.vector.tensor_mul(qpe, accs[2], ra)           # cos terms
            nc.vector.tensor_add(qpe, qpe, qtmp)

            if ci == S // CH - 1:
                # h1's q_pe rows live at partitions 64:128; matmul needs
                # lhsT/rhs on the same base partition, so shift to base 0.
                qpe1 = batchp.tile([64, S], BF16, tag="qpe1", name=f"qpe1{b}")
                nc.sync.dma_start(out=qpe1, in_=qT[64:128, 2, :])
                qpe1s.append(qpe1)
    pq_stack.close()

    # ============ Phase KV + attention, per batch ============
    pk_stack = ExitStack()
    pk = pk_stack.enter_context(tc.tile_pool(name="pk", bufs=3))
    kv_chunks = [(b, ci) for b in range(B) for ci in range(S // CH)]

    def latc_dma(b, ci):
        t = pk.tile([128, N_RT, CH], BF16, tag="latc", bufs=3, name=f"latc{b}{ci}")
        for j in range(CH // SLC):
            t0 = ci * CH + j * SLC
            nc.sync.dma_start(
                out=t[:, :, j * SLC:(j + 1) * SLC],
                in_=lat_g[t0 // SLC, b, 0:512, :].rearrange("(r p) s -> p r s", p=128))
        return t

    latc_pref = [latc_dma(*kv_chunks[0]), latc_dma(*kv_chunks[1])]

    for b in range(B):
        qT, qpe1 = qTs[b], qpe1s[b]
        knT = batchp.tile([128, HPC, S], BF16, tag="knT", bufs=1)
        kpT = batchp.tile([64, S], BF16, tag="kpT", bufs=2)
        vT = batchp.tile([128, N_KT, HPC * D_V], BF16, tag="vT", bufs=1)

        with tc.tile_pool(name="pkps", bufs=4, space="PSUM") as pkps:
            for ci in range(S // CH):
                c0 = ci * CH
                idx = b * (S // CH) + ci
                latc = latc_pref[idx]
                if idx + 2 < len(kv_chunks):
                    latc_pref.append(latc_dma(*kv_chunks[idx + 2]))
                for j in range(CH // SLC):
                    t0 = c0 + j * SLC
                    nc.sync.dma_start(out=kpT[:, t0:t0 + SLC],
                                      in_=lat_g[t0 // SLC, b, 512:576, :])
                # k_nope = wkb_k @ latn   [2 head tiles x CH]
                for m in range(HPC):
                    kn = pkps.tile([128, CH], F32, tag="kn")
                    for r in range(N_RT):
                        nc.tensor.matmul(kn, wkb_s[:, r, m * 128:(m + 1) * 128],
                                         latc[:, r, :], start=(r == 0),
                                         stop=(r == N_RT - 1))
                    nc.vector.tensor_copy(knT[:, m, c0:c0 + CH], kn)
                # v (token-major) = latn^T @ wkb_v   [CH/128 tiles x 256]
                for sti in range(CH // 128):
                    vp = pkps.tile([128, HPC * D_V], F32, tag="vp")
                    for r in range(N_RT):
                        nc.tensor.matmul(vp, latc[:, r, sti * 128:(sti + 1) * 128],
                                         wkb_s[:, r, 256:512], start=(r == 0),
                                         stop=(r == N_RT - 1))
                    nc.vector.tensor_copy(vT[:, c0 // 128 + sti, :], vp)

        # ---- attention + wo ----
        # Per chunk, issue order is arranged so the PE never waits on the
        # scalar/vector normalize chains: h0's reciprocal chain runs under
        # h1's score matmuls; h1's runs under the previous chunk's deferred
        # wo matmuls; chunk c's wo runs inside chunk c+1.
        with tc.tile_pool(name="pb", bufs=2) as pb, \
             tc.tile_pool(name="pbe", bufs=4) as pbe, \
             tc.tile_pool(name="pbf", bufs=3) as pbf, \
             tc.tile_pool(name="pbps", bufs=1, space="PSUM") as pbps:

            def emit_wo(pw):
                p_ohs, p_sq0 = pw
                for mo in range(N_DT):
                    ps_f = pbps.tile([128, SQC], F32, tag="fin", bufs=2,
                                     name=f"fin{mo}")
                    for h in range(HPC):
                        nc.tensor.matmul(ps_f, wo_s[:, h, mo * 128:(mo + 1) * 128],
                                         p_ohs[h], start=(h == 0),
                                         stop=(h == HPC - 1))
                    ft = pbf.tile([128, SQC], F16, tag="ft", name=f"ft{mo}")
                    eng = nc.vector if mo % 2 else nc.scalar
                    eng.tensor_copy(ft, ps_f) if mo % 2 else eng.copy(ft, ps_f)
                    nc.sync.dma_start(
                        out=o[b, mo * 128:(mo + 1) * 128, p_sq0:p_sq0 + SQC], in_=ft)

            def tail_pe(ps_out, denrow, h):
                # broadcast den to all partitions, reciprocal on 128 lanes
                ps_rd = pbps.tile([128, SQC], F32, tag="fin", bufs=2,
                                  name=f"rd{h}")
                nc.tensor.matmul(ps_rd, ones_row, denrow, start=True, stop=True)
                rdb = pb.tile([128, SQC], F32, tag=f"rdb{h}")
                nc.vector.reciprocal(rdb, ps_rd)
                oh = pb.tile([128, SQC], BF16, tag=f"oh{h}")
                nc.vector.tensor_mul(oh, ps_out, rdb)
                return oh

            pending_wo = None
            for c in range(S // SQC):
                sq0 = c * SQC
                kts = list(range(4 * (c + 1))) if mask_mode == "causal" \
                    else list(range(N_KT))
                tails = []

                def kt_loop(h, mid_hook=None):
                    ps_out = pbps.tile([128, SQC], F32, tag="out", bufs=2,
                                       name=f"out{h}")
                    ps_den = pbps.tile([1, SQC], F32, tag="den", bufs=2,
                                       name=f"den{h}")
                    qn = qT[:, h, sq0:sq0 + SQC]
                    qp = qT[0:64, 2, sq0:sq0 + SQC] if h == 0 \
                        else qpe1[:, sq0:sq0 + SQC]
                    pend = None  # (e, off, is_first, kt) pending den/out
                    hook = mid_hook
                    for kt in kts:
                        k0 = kt * 128
                        ps_st = pbps.tile([128, SQC], F32, tag="st", bufs=2)
                        e = pbe.tile([128, SQC], BF16, tag="expS")
                        off = 0
                        if mask_mode == "causal" and k0 >= sq0:
                            # diagonal-straddling block: only columns >= off
                            # live; earlier columns are first-touched by kt=0's
                            # full-range matmul, so partial-range accumulation
                            # into ps_den/ps_out stays correct via has_written.
                            off = k0 - sq0
                            nc.tensor.matmul(ps_st[:, off:], knT[:, h, k0:k0 + 128],
                                             qn[:, off:], start=True, stop=False)
                            nc.tensor.matmul(ps_st[:, off:], kpT[:, k0:k0 + 128],
                                             qp[:, off:], start=False, stop=True)
                            nc.scalar.activation(e[:, off:], ps_st[:, off:], EXP)
                            nc.vector.tensor_mul(e[:, off:off + 128],
                                                 e[:, off:off + 128], tri)
                        else:
                            nc.tensor.matmul(ps_st, knT[:, h, k0:k0 + 128], qn,
                                             start=True, stop=False)
                            nc.tensor.matmul(ps_st, kpT[:, k0:k0 + 128], qp,
                                             start=False, stop=True)
                            nc.scalar.activation(e, ps_st, EXP)
                            if mask_mode == "general":
                                em = pb.tile([128, SQC], BF16, tag="em")
                                nc.sync.dma_start(out=em,
                                                  in_=emaskT[kt, :, sq0:sq0 + SQC])
                                nc.vector.tensor_mul(e, e, em)
                        # den/out for the PREVIOUS kt: the exp above overlaps
                        # these matmuls instead of stalling the PE queue.
                        if pend is not None:
                            pe_, poff, pfirst, pkt = pend
                            nc.tensor.matmul(ps_den[:, poff:], ones_col,
                                             pe_[:, poff:], start=pfirst,
                                             stop=False, skip_group_check=True)
                            nc.tensor.matmul(ps_out[:, poff:],
                                             vT[:, pkt, h * 128:(h + 1) * 128],
                                             pe_[:, poff:], start=pfirst,
                                             stop=False, skip_group_check=True)
                            if hook is not None:
                                hook()
                                hook = None
                        pend = (e, off, kt == kts[0], kt)
                    pe_, poff, pfirst, pkt = pend
                    nc.tensor.matmul(ps_den[:, poff:], ones_col, pe_[:, poff:],
                                     start=pfirst, stop=True, skip_group_check=True)
                    nc.tensor.matmul(ps_out[:, poff:],
                                     vT[:, pkt, h * 128:(h + 1) * 128],
                                     pe_[:, poff:], start=pfirst, stop=True,
                                     skip_group_check=True)
                    if hook is not None:  # single-kt chunks never hit the hook
                        hook()
                    # kick the single-lane PSUM->SBUF den copy now (scalar
                    # queue) so the broadcast matmul never waits on it
                    denrow = pb.tile([1, SQC], BF16, tag=f"denrow{h}",
                                     name=f"denrow{h}")
                    nc.scalar.copy(denrow, ps_den)
                    return ps_out, denrow

                t0_ = kt_loop(0)
                oh0_box = []
                t1_ = kt_loop(1, mid_hook=lambda: oh0_box.append(tail_pe(*t0_, 0)))
                if pending_wo is not None:
                    emit_wo(pending_wo)
                    pending_wo = None
                oh1 = tail_pe(*t1_, 1)
                pending_wo = ([oh0_box[0], oh1], sq0)
            emit_wo(pending_wo)
    pk_stack.close()


def _mask_mode(mask):
    if not np.any(mask):
        return "none"
    iu = np.triu_indices(S, 1)
    upper = mask[iu]
    lower_ok = True
    il = np.tril_indices(S, 0)
    if not np.all(mask[il] == 0.0):
        lower_ok = False
    if lower_ok and np.all(np.isneginf(upper)):
        return "causal"
    return "general"


def _deint(rows):  # pack rope pairs: quadrant-local [even(16) | odd(16)] blocks
    return rows[_IDX64]


def _to_tiles16(mat):  # [K, M] -> [128, K/128, M] bf16 (partition-major k-tiles)
    k, m = mat.shape
    return np.ascontiguousarray(
        mat.reshape(k // 128, 128, m).transpose(1, 0, 2)).astype(NPBF16)


def kernel(x=None, start_pos=None, freqs_cis=None, mask=None, wq=None,
           wkv_a=None, wkv_b=None, wo=None, kv_norm_w=None, **_unused):
    x = np.asarray(x, dtype=np.float32)
    freqs_cis = np.asarray(freqs_cis, dtype=np.float32)
    mask = np.asarray(mask, dtype=np.float32)
    wq = np.asarray(wq, dtype=np.float32)
    wkv_a = np.asarray(wkv_a, dtype=np.float32)
    wkv_b = np.asarray(wkv_b, dtype=np.float32)
    wo = np.asarray(wo, dtype=np.float32)
    kv_norm_w = np.asarray(kv_norm_w, dtype=np.float32)

    mode = _mask_mode(mask)
    if mode not in _cache:
        _cache[mode] = _build(mode)
    nc = _cache[mode]

    scale = float(D_QK) ** -0.5
    xTarr = np.ascontiguousarray(
        x.reshape(B, S, N_DT, 128).transpose(0, 2, 3, 1)).astype(NPBF16)

    wka_perm = np.concatenate([wkv_a[:KV_RANK], _deint(wkv_a[KV_RANK:])], axis=0)
    wkaT_arr = _to_tiles16(wka_perm.T)  # [128, 16, 576]

    cos = freqs_cis[:, :, 0].T  # [32, S]
    sin = freqs_cis[:, :, 1].T
    a64 = np.concatenate([cos[0:16], cos[0:16], cos[16:32], cos[16:32]], axis=0)
    b64 = np.concatenate([-sin[0:16], sin[0:16], -sin[16:32], sin[16:32]], axis=0)
    ropeA_arr = np.ascontiguousarray(np.concatenate([a64, a64], axis=0))
    ropeB_arr = np.ascontiguousarray(np.concatenate([b64, b64], axis=0))
    tri_arr = np.triu(np.ones((128, 128), np.float32)).astype(NPBF16)

    emaskT_arr = None
    if mode == "general":
        em = np.exp(np.minimum(mask.T, 80.0)).astype(np.float32)  # [sk, sq]
        emaskT_arr = np.ascontiguousarray(em.reshape(N_KT, 128, S)).astype(NPBF16)

    wqh = wq.reshape(NH, D_QK, DIM)
    wkb_scaled = wkv_b * kv_norm_w[None, :]
    wkbh = wkb_scaled.reshape(NH, D_NOPE + D_V, KV_RANK)

    in_maps = []
    for cc in range(N_CORES):
        h0, h1 = HPC * cc, HPC * cc + 1
        pe0, pe1 = wqh[h0, D_NOPE:], wqh[h1, D_NOPE:]
        wq_c = np.concatenate(
            [wqh[h0, :D_NOPE], wqh[h1, :D_NOPE], _deint(pe0), _deint(pe1)], axis=0
        ) * scale  # [384, DIM]
        wkb_c = np.concatenate(
            [wkbh[h0, :D_NOPE], wkbh[h1, :D_NOPE], wkbh[h0, D_NOPE:], wkbh[h1, D_NOPE:]],
            axis=0,
        )  # [512, KV_RANK]
        wo_c = wo[:, HPC * cc * D_V:(HPC * cc + HPC) * D_V]  # [DIM, 256]
        sl = slice(cc * SLC, (cc + 1) * SLC)
        xsT_arr = np.ascontiguousarray(xTarr[:, :, :, sl])
        ropeS_sl_a = np.ascontiguousarray(np.tile(ropeA_arr[0:64, sl], (1, B)))
        ropeS_sl_b = np.ascontiguousarray(np.tile(ropeB_arr[0:64, sl], (1, B)))
        m = {
            "xT": xTarr,
            "xsT": xsT_arr,
            "wqT": _to_tiles16(wq_c.T),
            "wkaT": wkaT_arr,
            "wkbT": _to_tiles16(wkb_c.T),
            "woT": _to_tiles16(wo_c.T),
            "ropeA": ropeA_arr,
            "ropeB": ropeB_arr,
            "ropeSA": ropeS_sl_a,
            "ropeSB": ropeS_sl_b,
            "tri16": tri_arr,
        }
        if mode == "general":
            m["emaskT"] = emaskT_arr
        in_maps.append(m)

    res = bass_utils.run_bass_kernel_spmd(nc, in_maps, core_ids=list(range(N_CORES)))
    global last_results
    last_results = res
    out = res.results[0]["o"].astype(np.float32)
    for cc in range(1, N_CORES):
        out += res.results[cc]["o"].astype(np.float32)
    return np.ascontiguousarray(out.transpose(0, 2, 1)).astype(np.float32)


# revision 16
# speedup vs baseline: 1.0204x; 1.0204x over previous
"""MLA (multi-head latent attention) prefill kernel for 8 Trainium2 NeuronCores.

Tensor-parallel across heads: each of the 8 cores owns NH/8 = 2 heads.
wq / wkv_b output dims and the wo input dim are sharded by head; the post-wo
partial sums are reduced on the host (the unshard step of a RowParallelLinear).

The shared latent-KV projection (wkv_a) + rms-norm is NOT replicated: each
core computes it for a 256-token slice of the sequence (both batches fused
into one 512-column pass) and the normalized latent (+ rope'd k_pe) is
exchanged with a single on-device AllGather, cutting the dominant phase-A
matmul volume 8x versus the replicated form.

Everything on-device runs in a transposed [feature, seq] layout so that
attention scores come out as S^T[sk, sq]; the softmax reductions over the
key axis (= partitions) are done with ones-vector matmuls (denominator) and
a gpsimd partition_all_reduce (rms-norm sum of squares), so the kernel needs
no on-chip transposes. Max-subtraction is skipped (logits are O(10), exp is
safe in fp32). All matmul operands are bf16 (full PE rate, half the
LDWEIGHTS + SBUF + DMA cost of fp32r); PSUM accumulation stays fp32.
Reciprocals are taken after broadcasting to 128 partitions (a [1,N]
single-lane reciprocal is ~6x slower than a [128,N] one).

Scheduling notes: phase-Q input DMA is prefetched before the collective
competes for DMA bandwidth; softmax den/out matmuls run one kt behind the
exp that feeds them; each chunk's wo projection is deferred into the next
chunk's score matmuls so its normalize chain never stalls the PE.

Host-side prep: weights are pre-permuted so rope pairs are de-interleaved
([even | odd] blocks) and head blocks land on clean 128-partition tiles;
the 1/sqrt(d_qk) scale and kv_norm weight are folded into wq / wkv_b.
"""

import sys

sys.path.insert(0, "/opt/trn_rl_repo")

from contextlib import ExitStack

import numpy as np
import ml_dtypes

import concourse.tile as tile
from concourse import bacc, mybir
from concourse import bass_utils
from concourse import bass_isa

B, S, DIM = 2, 2048, 2048
NH = 16
D_NOPE, D_ROPE, D_V = 128, 64, 128
D_QK = D_NOPE + D_ROPE  # 192
KV_RANK = 512
RMS_EPS = 1e-6
N_CORES = 8
HPC = NH // N_CORES  # heads per core = 2

F32 = mybir.dt.float32
F32R = mybir.dt.float32r
BF16 = mybir.dt.bfloat16
F16 = mybir.dt.float16
EXP = mybir.ActivationFunctionType.Exp
SQRT = mybir.ActivationFunctionType.Sqrt

NPBF16 = ml_dtypes.bfloat16

CH = 512            # wq / kn-v seq chunk (moving N of projection matmuls)
SLC = S // N_CORES  # 256: per-core wkv_a token slice
SL2 = B * SLC       # 512: both batches' slices fused into one pass
SQC = 512           # phase-B query chunk
N_DT = DIM // 128   # 16 k-tiles over model dim
N_RT = KV_RANK // 128  # 4 k-tiles over kv rank
N_KT = S // 128     # 16 key tiles

# stream_shuffle permutes WITHIN each 32-partition quadrant (same mask per
# quadrant), so rope pairs are packed [even(16) | odd(16)] per quadrant and the
# shuffle swaps the 16-row halves.
SHUF_Q = list(range(16, 32)) + list(range(16))
SHUF_K = SHUF_Q

# row permutation packing a 64-row interleaved rope block into that layout:
# pair i -> even at 32*(i//16) + i%16, odd at 32*(i//16) + 16 + i%16
_IDX64 = [0] * 64
for _i in range(32):
    _IDX64[32 * (_i // 16) + (_i % 16)] = 2 * _i
    _IDX64[32 * (_i // 16) + 16 + (_i % 16)] = 2 * _i + 1

_cache = {}
last_results = None


def _build(mask_mode):
    nc = bacc.Bacc("TRN2", target_bir_lowering=False, debug=False, num_devices=N_CORES)

    xT = nc.dram_tensor("xT", [B, N_DT, 128, S], BF16, kind="ExternalInput").ap()
    xsT = nc.dram_tensor("xsT", [B, N_DT, 128, SLC], BF16, kind="ExternalInput").ap()
    wqT = nc.dram_tensor("wqT", [128, N_DT, 384], BF16, kind="ExternalInput").ap()
    wkaT = nc.dram_tensor("wkaT", [128, N_DT, 576], BF16, kind="ExternalInput").ap()
    wkbT = nc.dram_tensor("wkbT", [128, N_RT, 512], BF16, kind="ExternalInput").ap()
    woT = nc.dram_tensor("woT", [128, HPC, DIM], BF16, kind="ExternalInput").ap()
    ropeA = nc.dram_tensor("ropeA", [128, S], F32R, kind="ExternalInput").ap()
    ropeB = nc.dram_tensor("ropeB", [128, S], F32R, kind="ExternalInput").ap()
    ropeSA = nc.dram_tensor("ropeSA", [64, SL2], F32R, kind="ExternalInput").ap()
    ropeSB = nc.dram_tensor("ropeSB", [64, SL2], F32R, kind="ExternalInput").ap()
    tri16 = nc.dram_tensor("tri16", [128, 128], BF16, kind="ExternalInput").ap()
    emaskT = None
    if mask_mode == "general":
        emaskT = nc.dram_tensor("emaskT", [N_KT, 128, S], BF16, kind="ExternalInput").ap()
    o = nc.dram_tensor("o", [B, DIM, S], F16, kind="ExternalOutput").ap()

    with tile.TileContext(nc) as tc:
        with ExitStack() as ctx, \
                nc.allow_low_precision(reason="bf16 matmul pipeline"):
            _body(ctx, tc, mask_mode, xT, xsT, wqT, wkaT, wkbT, woT,
                  ropeA, ropeB, ropeSA, ropeSB, tri16, emaskT, o)
    nc.compile()
    return nc


def _body(ctx, tc, mask_mode, xT, xsT, wqT, wkaT, wkbT, woT,
          ropeA, ropeB, ropeSA, ropeSB, tri16, emaskT, o):
    nc = tc.nc

    singles = ctx.enter_context(tc.tile_pool(name="singles", bufs=1))
    wka_s = singles.tile([128, N_DT, 576], BF16)
    nc.sync.dma_start(out=wka_s, in_=wkaT)
    wq_s = singles.tile([128, N_DT, 384], BF16)
    nc.sync.dma_start(out=wq_s, in_=wqT)
    # wkb/wo are not needed until phase KV; their DMAs are issued after the
    # latent slice ships so the collective is never queued behind them
    wkb_s = singles.tile([128, N_RT, 512], BF16)
    wo_s = singles.tile([128, HPC, DIM], BF16)
    cst = singles.tile([128, 128], BF16)
    nc.sync.dma_start(out=cst, in_=tri16)
    ones_col = cst[:, 127:128]   # [128,1] lhsT -> partition sum
    ones_row = cst[0:1, 0:128]   # [1,128] lhsT -> partition broadcast
    tri = cst[:, 0:128]          # keep-mask: 1 where free >= part
    rsa_s = singles.tile([64, SL2], F32R)
    nc.sync.dma_start(out=rsa_s, in_=ropeSA)
    rsb_s = singles.tile([64, SL2], F32R)
    nc.sync.dma_start(out=rsb_s, in_=ropeSB)
    epsb = singles.tile([128, 1], F32)
    nc.vector.memset(epsb, RMS_EPS)

    batchp = ctx.enter_context(tc.tile_pool(name="batchp", bufs=2))
    dramp = ctx.enter_context(tc.tile_pool(name="dramp", bufs=1, space="DRAM"))
    # xc pool opened before phase S so phase-Q input DMA is prefetched before
    # the collective starts competing for DMA bandwidth; closed after phase Q
    pq_stack = ExitStack()
    pq = pq_stack.enter_context(tc.tile_pool(name="pq", bufs=3))
    pq1 = pq_stack.enter_context(tc.tile_pool(name="pq1", bufs=2))

    all_chunks = [(b, ci) for b in range(B) for ci in range(S // CH)]

    def xc_dma(b, ci):
        t = pq.tile([128, N_DT, CH], BF16, tag="xc", bufs=3, name=f"xc{b}{ci}")
        nc.sync.dma_start(
            out=t, in_=xT[b, :, :, ci * CH:(ci + 1) * CH].rearrange("t p s -> p t s"))
        return t

    xc_pref = [xc_dma(*all_chunks[0]), xc_dma(*all_chunks[1])]

    # ========== Phase S: latent-kv slice (both batches fused) + gather =====
    lat_in = dramp.tile([B, 576, SLC], BF16, tag="lat_in")
    lat_g = dramp.tile([N_CORES, B, 576, SLC], BF16, tag="lat_g",
                       addr_space="Shared")

    with tc.tile_pool(name="psl", bufs=1) as psl, \
         tc.tile_pool(name="pslps", bufs=1, space="PSUM") as pslps:
        xs = psl.tile([128, N_DT, SL2], BF16)
        for b in range(B):
            nc.sync.dma_start(
                out=xs[:, :, b * SLC:(b + 1) * SLC],
                in_=xsT[b].rearrange("t p s -> p t s"))
        accs = [pslps.tile([128, SL2], F32, tag=f"acc{i}", bufs=1, name=f"acc{i}")
                for i in range(N_RT)]
        acc_pe = pslps.tile([64, SL2], F32, tag="accpe", bufs=1)
        for dt in range(N_DT):
            st, sp = dt == 0, dt == N_DT - 1
            for m in range(N_RT):
                nc.tensor.matmul(accs[m], wka_s[:, dt, m * 128:(m + 1) * 128],
                                 xs[:, dt, :], start=st, stop=sp)
            nc.tensor.matmul(acc_pe, wka_s[:, dt, 512:576],
                             xs[:, dt, :], start=st, stop=sp)

        # squares straight off PSUM feed the rms reduce ASAP (scalar);
        # the latent copies run concurrently on the vector engine so the
        # acc banks free for phase Q without waiting on the norm chain
        sq = psl.tile([128, N_RT, SL2], BF16)
        for m in range(N_RT):
            nc.scalar.square(sq[:, m, :], accs[m])
        kvl = psl.tile([128, N_RT, SL2], BF16)
        for m in range(N_RT):
            nc.vector.tensor_copy(kvl[:, m, :], accs[m])
        # k_pe rope (64 rows) — independent of the rms norm
        ktmp = psl.tile([64, SL2], F32)
        kpe16 = psl.tile([64, SL2], BF16)
        nc.vector.stream_shuffle(ktmp, acc_pe, SHUF_K)
        nc.vector.tensor_mul(ktmp, ktmp, rsb_s)
        nc.vector.tensor_mul(kpe16, acc_pe, rsa_s)
        nc.vector.tensor_add(kpe16, kpe16, ktmp)

        # rms-norm: sum of squares across the 512 latent rows via a
        # ones-matmul partition reduce, then a broadcast so the reciprocal
        # runs on all 128 lanes (ss and bcd share one PSUM bank via WAR)
        ss = pslps.tile([1, SL2], F32, tag="red", bufs=1, name="ss")
        for m in range(N_RT):
            nc.tensor.matmul(ss, ones_col, sq[:, m, :],
                             start=(m == 0), stop=(m == N_RT - 1))
        mrow = psl.tile([1, SL2], BF16)
        nc.scalar.activation(mrow, ss, SQRT, bias=epsb[0:1, :],
                             scale=1.0 / KV_RANK)
        bcd = pslps.tile([128, SL2], F32, tag="red", bufs=1, name="bcd")
        nc.tensor.matmul(bcd, ones_row, mrow, start=True, stop=True)
        rsb = psl.tile([128, SL2], F32R)
        nc.vector.reciprocal(rsb, bcd)
        latn = psl.tile([128, N_RT, SL2], BF16)
        for m in range(N_RT):
            nc.vector.tensor_mul(latn[:, m, :], kvl[:, m, :], rsb)

        for b in range(B):
            nc.sync.dma_start(
                out=lat_in[b, 0:512, :].rearrange("(r p) s -> p r s", p=128),
                in_=latn[:, :, b * SLC:(b + 1) * SLC])
            nc.sync.dma_start(out=lat_in[b, 512:576, :],
                              in_=kpe16[:, b * SLC:(b + 1) * SLC])
    nc.gpsimd.collective_compute(
        "AllGather",
        mybir.AluOpType.bypass,
        replica_groups=[list(range(N_CORES))],
        ins=[lat_in.opt()],
        outs=[lat_g.opt()],
    )
    nc.sync.dma_start(out=wkb_s, in_=wkbT)
    nc.sync.dma_start(out=wo_s, in_=woT)

    # ================= Phase Q: query projection =================
    qTs, qpe1s = [], []
    with tc.tile_pool(name="pqps", bufs=6, space="PSUM") as pqps:
        qT = None
        for idx, (b, ci) in enumerate(all_chunks):
            if ci == 0:
                qT = batchp.tile([128, 3, S], BF16, tag="qT", name=f"qT{b}")
                qTs.append(qT)
            c0 = ci * CH
            xc = xc_pref[idx]
            if idx + 2 < len(all_chunks):
                xc_pref.append(xc_dma(*all_chunks[idx + 2]))
            ra = pq1.tile([128, CH], F32R, tag="ra", name=f"ra{b}{ci}")
            nc.sync.dma_start(out=ra, in_=ropeA[:, c0:c0 + CH])
            rb = pq1.tile([128, CH], F32R, tag="rb", name=f"rb{b}{ci}")
            nc.sync.dma_start(out=rb, in_=ropeB[:, c0:c0 + CH])

            accs = [pqps.tile([128, CH], F32, tag="wq", name=f"wq{i}")
                    for i in range(3)]
            for dt in range(N_DT):
                st, sp = dt == 0, dt == N_DT - 1
                for m in range(3):
                    nc.tensor.matmul(accs[m], wq_s[:, dt, m * 128:(m + 1) * 128],
                                     xc[:, dt, :], start=st, stop=sp)
            for m in range(2):
                nc.vector.tensor_copy(qT[:, m, c0:c0 + CH], accs[m])
            qpe = qT[:, 2, c0:c0 + CH]
            qtmp = pq1.tile([128, CH], F32, tag="qtmp", name=f"qtmp{b}{ci}")
            nc.vector.stream_shuffle(qtmp, accs[2], SHUF_Q)  # [o|e] swapped
            nc.vector.tensor_mul(qtmp, qtmp, rb)             # +/- sin terms
            nc.vector.tensor_mul(qpe, accs[2], ra)           # cos terms
            nc.vector.tensor_add(qpe, qpe, qtmp)

            if ci == S // CH - 1:
                # h1's q_pe rows live at partitions 64:128; matmul needs
                # lhsT/rhs on the same base partition, so shift to base 0.
                qpe1 = batchp.tile([64, S], BF16, tag="qpe1", name=f"qpe1{b}")
                nc.sync.dma_start(out=qpe1, in_=qT[64:128, 2, :])
                qpe1s.append(qpe1)
    pq_stack.close()

    # ============ Phase KV + attention, per batch ============
    pk_stack = ExitStack()
    pk = pk_stack.enter_context(tc.tile_pool(name="pk", bufs=3))
    kv_chunks = [(b, ci) for b in range(B) for ci in range(S // CH)]

    def latc_dma(b, ci):
        t = pk.tile([128, N_RT, CH], BF16, tag="latc", bufs=3, name=f"latc{b}{ci}")
        for j in range(CH // SLC):
            t0 = ci * CH + j * SLC
            nc.sync.dma_start(
                out=t[:, :, j * SLC:(j + 1) * SLC],
                in_=lat_g[t0 // SLC, b, 0:512, :].rearrange("(r p) s -> p r s", p=128))
        return t

    latc_pref = [latc_dma(*kv_chunks[0]), latc_dma(*kv_chunks[1])]

    for b in range(B):
        qT, qpe1 = qTs[b], qpe1s[b]
        knT = batchp.tile([128, HPC, S], BF16, tag="knT", bufs=1)
        kpT = batchp.tile([64, S], BF16, tag="kpT", bufs=2)
        vT = batchp.tile([128, N_KT, HPC * D_V], BF16, tag="vT", bufs=1)

        with tc.tile_pool(name="pkps", bufs=4, space="PSUM") as pkps:
            for ci in range(S // CH):
                c0 = ci * CH
                idx = b * (S // CH) + ci
                latc = latc_pref[idx]
                if idx + 2 < len(kv_chunks):
                    latc_pref.append(latc_dma(*kv_chunks[idx + 2]))
                for j in range(CH // SLC):
                    t0 = c0 + j * SLC
                    nc.sync.dma_start(out=kpT[:, t0:t0 + SLC],
                                      in_=lat_g[t0 // SLC, b, 512:576, :])
                # k_nope = wkb_k @ latn   [2 head tiles x CH]
                for m in range(HPC):
                    kn = pkps.tile([128, CH], F32, tag="kn")
                    for r in range(N_RT):
                        nc.tensor.matmul(kn, wkb_s[:, r, m * 128:(m + 1) * 128],
                                         latc[:, r, :], start=(r == 0),
                                         stop=(r == N_RT - 1))
                    nc.vector.tensor_copy(knT[:, m, c0:c0 + CH], kn)
                # v (token-major) = latn^T @ wkb_v   [CH/128 tiles x 256]
                for sti in range(CH // 128):
                    vp = pkps.tile([128, HPC * D_V], F32, tag="vp")
                    for r in range(N_RT):
                        nc.tensor.matmul(vp, latc[:, r, sti * 128:(sti + 1) * 128],
                                         wkb_s[:, r, 256:512], start=(r == 0),
                                         stop=(r == N_RT - 1))
                    nc.vector.tensor_copy(vT[:, c0 // 128 + sti, :], vp)

        # ---- attention + wo ----
        # Per chunk, issue order is arranged so the PE never waits on the
        # scalar/vector normalize chains: h0's reciprocal chain runs under
        # h1's score matmuls; h1's runs under the previous chunk's deferred
        # wo matmuls; chunk c's wo runs inside chunk c+1.
        with tc.tile_pool(name="pb", bufs=2) as pb, \
             tc.tile_pool(name="pbe", bufs=4) as pbe, \
             tc.tile_pool(name="pbf", bufs=3) as pbf, \
             tc.tile_pool(name="pbps", bufs=1, space="PSUM") as pbps:

            def emit_wo(pw):
                p_ohs, p_sq0 = pw
                for mo in range(N_DT):
                    ps_f = pbps.tile([128, SQC], F32, tag="fin", bufs=2,
                                     name=f"fin{mo}")
                    for h in range(HPC):
                        nc.tensor.matmul(ps_f, wo_s[:, h, mo * 128:(mo + 1) * 128],
                                         p_ohs[h], start=(h == 0),
                                         stop=(h == HPC - 1))
                    ft = pbf.tile([128, SQC], F16, tag="ft", name=f"ft{mo}")
                    eng = nc.vector if mo % 2 else nc.scalar
                    eng.tensor_copy(ft, ps_f) if mo % 2 else eng.copy(ft, ps_f)
                    nc.sync.dma_start(
                        out=o[b, mo * 128:(mo + 1) * 128, p_sq0:p_sq0 + SQC], in_=ft)

            def tail_pe(ps_out, denrow, h):
                # broadcast den to all partitions, reciprocal on 128 lanes
                ps_rd = pbps.tile([128, SQC], F32, tag="fin", bufs=2,
                                  name=f"rd{h}")
                nc.tensor.matmul(ps_rd, ones_row, denrow, start=True, stop=True)
                rdb = pb.tile([128, SQC], F32, tag=f"rdb{h}")
                nc.vector.reciprocal(rdb, ps_rd)
                oh = pb.tile([128, SQC], BF16, tag=f"oh{h}")
                nc.vector.tensor_mul(oh, ps_out, rdb)
                return oh

            pending_wo = None
            for c in range(S // SQC):
                sq0 = c * SQC
                kts = list(range(4 * (c + 1))) if mask_mode == "causal" \
                    else list(range(N_KT))
                # one merged (h, kt) stream: den/out matmuls run one item
                # behind the exp that feeds them, including across the h0->h1
                # boundary; h1's final flush is covered by chunk c-1's wo
                items = [(h, kt) for h in range(HPC) for kt in kts]
                pso = {}, {}
                ps_outs, denrows, ohs = {}, {}, {}
                for h in range(HPC):
                    ps_outs[h] = (
                        pbps.tile([128, SQC], F32, tag="out", bufs=2,
                                  name=f"out{h}"),
                        pbps.tile([1, SQC], F32, tag="den", bufs=2,
                                  name=f"den{h}"))
                qps = [qT[0:64, 2, sq0:sq0 + SQC], qpe1[:, sq0:sq0 + SQC]]

                def emit_pend(pend, stop):
                    ph, pe_, poff, pfirst, pkt = pend
                    ps_out, ps_den = ps_outs[ph]
                    nc.tensor.matmul(ps_den[:, poff:], ones_col, pe_[:, poff:],
                                     start=pfirst, stop=stop,
                                     skip_group_check=True)
                    nc.tensor.matmul(ps_out[:, poff:],
                                     vT[:, pkt, ph * 128:(ph + 1) * 128],
                                     pe_[:, poff:], start=pfirst, stop=stop,
                                     skip_group_check=True)
                    if stop:
                        # kick the single-lane PSUM->SBUF den copy on the
                        # scalar queue right away
                        denrow = pb.tile([1, SQC], BF16, tag=f"denrow{ph}",
                                         name=f"denrow{ph}")
                        nc.scalar.copy(denrow, ps_den)
                        denrows[ph] = denrow

                pend = None
                h0_tail_after = None
                for i, (h, kt) in enumerate(items):
                    k0 = kt * 128
                    ps_st = pbps.tile([128, SQC], F32, tag="st", bufs=2)
                    e = pbe.tile([128, SQC], BF16, tag="expS")
                    qn = qT[:, h, sq0:sq0 + SQC]
                    qp = qps[h]
                    off = 0
                    if mask_mode == "causal" and k0 >= sq0:
                        # diagonal-straddling block: only columns >= off live;
                        # earlier columns are first-touched by kt=0's
                        # full-range matmul, so partial-range accumulation
                        # into ps_den/ps_out stays correct via has_written.
                        off = k0 - sq0
                        nc.tensor.matmul(ps_st[:, off:], knT[:, h, k0:k0 + 128],
                                         qn[:, off:], start=True, stop=False)
                        nc.tensor.matmul(ps_st[:, off:], kpT[:, k0:k0 + 128],
                                         qp[:, off:], start=False, stop=True)
                        nc.scalar.activation(e[:, off:], ps_st[:, off:], EXP)
                        nc.vector.tensor_mul(e[:, off:off + 128],
                                             e[:, off:off + 128], tri)
                    else:
                        nc.tensor.matmul(ps_st, knT[:, h, k0:k0 + 128], qn,
                                         start=True, stop=False)
                        nc.tensor.matmul(ps_st, kpT[:, k0:k0 + 128], qp,
                                         start=False, stop=True)
                        nc.scalar.activation(e, ps_st, EXP)
                        if mask_mode == "general":
                            em = pb.tile([128, SQC], BF16, tag="em")
                            nc.sync.dma_start(out=em,
                                              in_=emaskT[kt, :, sq0:sq0 + SQC])
                            nc.vector.tensor_mul(e, e, em)
                    if pend is not None:
                        emit_pend(pend, stop=(pend[0] != h))
                        if pend[0] != h:
                            # h0 fully flushed: its normalize chain runs
                            # under h1's score matmuls, two items later
                            h0_tail_after = i + 1
                    if h0_tail_after == i:
                        ohs[0] = tail_pe(*(ps_outs[0][0], denrows[0]), 0)
                    pend = (h, e, off, kt == kts[0], kt)
                if pending_wo is not None:
                    # the 32 wo matmuls of chunk c-1 cover the last exp
                    emit_wo(pending_wo)
                    pending_wo = None
                emit_pend(pend, stop=True)
                if h0_tail_after is not None and 0 not in ohs:
                    ohs[0] = tail_pe(*(ps_outs[0][0], denrows[0]), 0)
                ohs[1] = tail_pe(*(ps_outs[1][0], denrows[1]), 1)
                pending_wo = ([ohs[0], ohs[1]], sq0)
            emit_wo(pending_wo)
    pk_stack.close()


def _mask_mode(mask):
    if not np.any(mask):
        return "none"
    iu = np.triu_indices(S, 1)
    upper = mask[iu]
    lower_ok = True
    il = np.tril_indices(S, 0)
    if not np.all(mask[il] == 0.0):
        lower_ok = False
    if lower_ok and np.all(np.isneginf(upper)):
        return "causal"
    return "general"


def _deint(rows):  # pack rope pairs: quadrant-local [even(16) | odd(16)] blocks
    return rows[_IDX64]


def _to_tiles16(mat):  # [K, M] -> [128, K/128, M] bf16 (partition-major k-tiles)
    k, m = mat.shape
    return np.ascontiguousarray(
        mat.reshape(k // 128, 128, m).transpose(1, 0, 2)).astype(NPBF16)


def kernel(x=None, start_pos=None, freqs_cis=None, mask=None, wq=None,
           wkv_a=None, wkv_b=None, wo=None, kv_norm_w=None, **_unused):
    x = np.asarray(x, dtype=np.float32)
    freqs_cis = np.asarray(freqs_cis, dtype=np.float32)
    mask = np.asarray(mask, dtype=np.float32)
    wq = np.asarray(wq, dtype=np.float32)
    wkv_a = np.asarray(wkv_a, dtype=np.float32)
    wkv_b = np.asarray(wkv_b, dtype=np.float32)
    wo = np.asarray(wo, dtype=np.float32)
    kv_norm_w = np.asarray(kv_norm_w, dtype=np.float32)

    mode = _mask_mode(mask)
    if mode not in _cache:
        _cache[mode] = _build(mode)
    nc = _cache[mode]

    scale = float(D_QK) ** -0.5
    xTarr = np.ascontiguousarray(
        x.reshape(B, S, N_DT, 128).transpose(0, 2, 3, 1)).astype(NPBF16)

    wka_perm = np.concatenate([wkv_a[:KV_RANK], _deint(wkv_a[KV_RANK:])], axis=0)
    wkaT_arr = _to_tiles16(wka_perm.T)  # [128, 16, 576]

    cos = freqs_cis[:, :, 0].T  # [32, S]
    sin = freqs_cis[:, :, 1].T
    a64 = np.concatenate([cos[0:16], cos[0:16], cos[16:32], cos[16:32]], axis=0)
    b64 = np.concatenate([-sin[0:16], sin[0:16], -sin[16:32], sin[16:32]], axis=0)
    ropeA_arr = np.ascontiguousarray(np.concatenate([a64, a64], axis=0))
    ropeB_arr = np.ascontiguousarray(np.concatenate([b64, b64], axis=0))
    tri_arr = np.triu(np.ones((128, 128), np.float32)).astype(NPBF16)

    emaskT_arr = None
    if mode == "general":
        em = np.exp(np.minimum(mask.T, 80.0)).astype(np.float32)  # [sk, sq]
        emaskT_arr = np.ascontiguousarray(em.reshape(N_KT, 128, S)).astype(NPBF16)

    wqh = wq.reshape(NH, D_QK, DIM)
    wkb_scaled = wkv_b * kv_norm_w[None, :]
    wkbh = wkb_scaled.reshape(NH, D_NOPE + D_V, KV_RANK)

    in_maps = []
    for cc in range(N_CORES):
        h0, h1 = HPC * cc, HPC * cc + 1
        pe0, pe1 = wqh[h0, D_NOPE:], wqh[h1, D_NOPE:]
        wq_c = np.concatenate(
            [wqh[h0, :D_NOPE], wqh[h1, :D_NOPE], _deint(pe0), _deint(pe1)], axis=0
        ) * scale  # [384, DIM]
        wkb_c = np.concatenate(
            [wkbh[h0, :D_NOPE], wkbh[h1, :D_NOPE], wkbh[h0, D_NOPE:], wkbh[h1, D_NOPE:]],
            axis=0,
        )  # [512, KV_RANK]
        wo_c = wo[:, HPC * cc * D_V:(HPC * cc + HPC) * D_V]  # [DIM, 256]
        sl = slice(cc * SLC, (cc + 1) * SLC)
        xsT_arr = np.ascontiguousarray(xTarr[:, :, :, sl])
        ropeS_sl_a = np.ascontiguousarray(np.tile(ropeA_arr[0:64, sl], (1, B)))
        ropeS_sl_b = np.ascontiguousarray(np.tile(ropeB_arr[0:64, sl], (1, B)))
        m = {
            "xT": xTarr,
            "xsT": xsT_arr,
            "wqT": _to_tiles16(wq_c.T),
            "wkaT": wkaT_arr,
            "wkbT": _to_tiles16(wkb_c.T),
            "woT": _to_tiles16(wo_c.T),
            "ropeA": ropeA_arr,
            "ropeB": ropeB_arr,
            "ropeSA": ropeS_sl_a,
            "ropeSB": ropeS_sl_b,
            "tri16": tri_arr,
        }
        if mode == "general":
            m["emaskT"] = emaskT_arr
        in_maps.append(m)

    res = bass_utils.run_bass_kernel_spmd(nc, in_maps, core_ids=list(range(N_CORES)))
    global last_results
    last_results = res
    out = res.results[0]["o"].astype(np.float32)
    for cc in range(1, N_CORES):
        out += res.results[cc]["o"].astype(np.float32)
    return np.ascontiguousarray(out.transpose(0, 2, 1)).astype(np.float32)
